# revision 16
# baseline (speedup 1.0000x reference)
"""Multi-head attention (B=4, S=2048, H=1024, NH=16) on 8 TRN2 NeuronCores.

Sharding: data-parallel over batch (4) x tensor-parallel over heads (2 groups
of 8 heads). Core c handles batch c//2, head-group c%2 (features 512*(c%2)..).
The host pre-transposes x to x^T [H, S] (bf16) and pre-packs W into the exact
SBUF layout [pair, 128, (chunk d)] so every weight load is one large
contiguous DMA; Q^T/K^T are kept in float32r (full PE rate, moving dim >=
256).

Per-core kernel:
  1. Projections run as fine-grained "units" (one PSUM accumulation group
     each) that are pumped matmul-by-matmul into the emission stream between
     attention chunks, so the in-order PE pipeline always has ready filler
     work while softmax-exp results are in flight.
  2. Attention per head-pair p (2 heads), 512-token q-block, 128-token
     kt-chunk:
       - two row-tiled QK^T matmuls produce S^T [128 kt, 512 q] per head,
       - exp(S^T/8 + mask) runs on EITHER ScalarE (activation) OR the DVE via
         a two-stage custom-DVE op (exp(t) = (((1+t*2^-11)^2+1)/2)^(2^11):
         base quadratic + 11 squarings split across two 8-stage uop passes),
         splitting the softmax-exp load across both engines,
       - PV matmuls are emitted in the [q, d] orientation: for each 128-token
         q-subtile j and head h, ctx[q, 0:64]+den[q] accumulates as
         pt[:, subtile]^T @ [V|ones] -- the 65-wide moving operand makes each
         accumulation step ~4x cheaper on the PE than the [d, q] orientation,
         and the softmax denominator still falls out of the ones column.
         PV emission trails QK by KLOOK chunks (software pipelining).
       - the 8 ctx streams of one (p, q-block) share two PSUM banks as one
         accumulation group each (PSUM pending-zero arms the whole 2KB region
         at the first start=True; each stream's first write then initializes
         its own bytes).
  3. ctx+den tiles drain via ScalarE copies to SBUF staging and DMA to DRAM;
     the host does the final normalization and layout (cheap numpy ops).
"""

import dataclasses
import os
from contextlib import ExitStack

import numpy as np

import concourse.mybir as mybir
import concourse.tile as tile
from concourse import bacc
from concourse.bass_utils import run_bass_kernel_spmd

B, S, H, NH, HD = 4, 2048, 1024, 16, 64
NCORES = 8
DP, TP = 4, 2            # batch-parallel x head-group-parallel
HG = NH // TP            # 8 heads per core
DG = HG * HD             # 512 features per core
NPAIR = HG // 2          # 4 head pairs (128 features each)
CCH = H // 128           # 8 contraction chunks for projections
TB = S // 512            # 4 token blocks of 512
TCH = S // 128           # 16 token chunks of 128
QB = S // 512            # 4 q-blocks of 512
NSTREAM = 8              # ctx accumulation streams per (pair, q-block)
F32 = mybir.dt.float32
F32R = mybir.dt.float32r
BF16 = mybir.dt.bfloat16

_CACHED = None
LAST_RESULTS = None   # BassKernelResults of the most recent run (for test.py)
TRACE = False         # set True (or BASS_KERNEL_TRACE=1) to profile the run

# ---------------------------------------------------------------------------
# Custom DVE exp: exp(t) ~= (0.5*(1 + t/2^11)^2 + 0.5)^(2^11), evaluated as
# pass1 = B(u)^(2^3) (base + 3 squarings, 8 ALU stages) and pass2 = x^(2^8)
# (8 squarings). Max rel err ~4e-4 in fp32 -- below bf16 output rounding.
# The mask enters as the per-partition scalar C1 = 1 + mask*2^-11.
# ---------------------------------------------------------------------------

_EXP_SCALE = float(0.125 * 2.0 ** -11)   # score scale 1/8 folded with 2^-11
_DVE_OPS = None


def _register_dve_exp():
    global _DVE_OPS
    if _DVE_OPS is not None:
        return _DVE_OPS
    import concourse.dve_ops as dve_ops
    from concourse.dve_ops import DveOp, _CUSTOM_DVE_ROW_BASE
    from concourse.dve_spec import Spec, Src0, C0, C1, C2, sq, lower
    from concourse.dve_spec import _has_src1 as has_src1
    from concourse.dve_uop import DveOpSpec

    def reg(name, spec):
        if name in dve_ops._SUB_OPCODE_FOR_NAME:
            return next(o for o in dve_ops.OPS if o.name == name)
        op = DveOp(name, spec, subdim=False, uops_sha={})
        dve_ops.OPS.append(op)
        dve_ops.CUSTOM_DVE_SPECS[name] = spec
        dve_ops._SUB_OPCODE_FOR_NAME[name] = _CUSTOM_DVE_ROW_BASE + len(dve_ops.OPS) - 1
        assert dve_ops._SUB_OPCODE_FOR_NAME[name] < 0x20, "opcode row overflow"
        shas = {}
        for ver in ("v3", "v4"):
            s = DveOpSpec(
                name=name,
                opcode=dve_ops._SUB_OPCODE_FOR_NAME[name],
                uops=lower(spec, ver=ver),
                rd1_en=has_src1(spec),
            )
            shas[ver] = s.sha(ver)
        op2 = dataclasses.replace(op, uops_sha=shas)
        dve_ops.OPS[-1] = op2
        return op2

    body1 = sq(Src0 * C0 + C1) * C2 + C2
    for _ in range(3):
        body1 = sq(body1)

    def ref1(in0, in1, s0, s1, imm2):
        x = in0.astype(np.float32)
        v = x * np.float32(s0) + np.float32(s1)
        y = ((v * v) * np.float32(imm2) + np.float32(imm2)).astype(np.float32)
        for _ in range(3):
            y = (y * y).astype(np.float32)
        return y

    body2 = Src0
    for _ in range(8):
        body2 = sq(body2)

    def ref2(in0, in1, s0, s1, imm2):
        y = in0.astype(np.float32)
        for _ in range(8):
            y = (y * y).astype(np.float32)
        return y

    p1 = reg("ANT_EXP_P1", Spec(body=body1, reference=ref1))
    p2 = reg("ANT_EXP_P2", Spec(body=body2, reference=ref2))
    _DVE_OPS = (p1, p2)
    return _DVE_OPS


def _build_core_program():
    nc = bacc.Bacc(
        "TRN2", target_bir_lowering=False, debug=False, enable_asserts=False
    )

    xqT = nc.declare_dram_parameter("xqT", [H, S], BF16, isOutput=False)
    xkT = nc.declare_dram_parameter("xkT", [H, S], BF16, isOutput=False)
    xvT = nc.declare_dram_parameter("xvT", [H, S], BF16, isOutput=False)
    # weights pre-packed per pair in SBUF layout: rows 128p+f, cols c*128+d
    wqP = nc.declare_dram_parameter("wqP", [NPAIR * 128, CCH * 128], BF16, isOutput=False)
    wkP = nc.declare_dram_parameter("wkP", [NPAIR * 128, CCH * 128], BF16, isOutput=False)
    # V weights stay [input-feature, output-feature] (moving operand is x)
    wvT = nc.declare_dram_parameter("wvT", [H, DG], BF16, isOutput=False)
    # combined per-partition constants: bq | bk | mask | maskp
    cb = nc.declare_dram_parameter("cb", [128, 2 * NPAIR + 2 * TCH], F32, isOutput=False)
    bv = nc.declare_dram_parameter("bv", [1, DG], BF16, isOutput=False)
    out = nc.declare_dram_parameter(
        "out", [NPAIR * QB * 128, NSTREAM * 65], F32, isOutput=True
    )

    with tile.TileContext(nc) as tc:
        _emit(tc, nc, xqT, xkT, xvT, wqP, wkP, wvT, cb, bv, out)

    nc.compile()
    return nc


def _emit(tc, nc, xqT, xkT, xvT, wqP, wkP, wvT, cb, bv, out):
    Exp = mybir.ActivationFunctionType.Exp

    use_dve = os.environ.get("KDVE", "1") == "1"
    dve_num = int(os.environ.get("KDVE_NUM", "4"))
    dve_den = int(os.environ.get("KDVE_DEN", "16"))
    LOOK = int(os.environ.get("KLOOK", "3"))
    MARGIN = int(os.environ.get("KMARGIN", "8"))
    PUMP_STEPS = int(os.environ.get("KPUMP", "3"))
    if use_dve:
        EXP_P1, EXP_P2 = _register_dve_exp()

    pools = ExitStack()
    const = pools.enter_context(tc.tile_pool(name="const", bufs=1))
    persist = pools.enter_context(tc.tile_pool(name="persist", bufs=1))
    xpool = pools.enter_context(tc.tile_pool(name="xpool", bufs=2))
    work = pools.enter_context(tc.tile_pool(name="work", bufs=6))
    y1pool = pools.enter_context(tc.tile_pool(name="y1pool", bufs=3))
    stg = pools.enter_context(tc.tile_pool(name="stg", bufs=2))
    psum = pools.enter_context(tc.tile_pool(name="psum", bufs=1, space="PSUM"))

    # ---- constants ----
    ones_row = const.tile([1, 128], BF16, tag="ones_row")
    nc.gpsimd.memset(ones_row[:], 1.0)
    zero_row = const.tile([1, 512], BF16, tag="zero_row")
    nc.gpsimd.memset(zero_row[:], 0.0)

    cb_sb = const.tile([128, 2 * NPAIR + 2 * TCH], F32, tag="cb")
    bq_sb = cb_sb[:, 0:NPAIR]
    bk_sb = cb_sb[:, NPAIR : 2 * NPAIR]
    mask_sb = cb_sb[:, 2 * NPAIR : 2 * NPAIR + TCH]
    maskp_sb = cb_sb[:, 2 * NPAIR + TCH : 2 * NPAIR + 2 * TCH]
    bv_sb = const.tile([1, DG], BF16, tag="bv")

    # ---- weights: per-pair lazy single-DMA loads ----
    # layout [128, (name-pair) * CCH*128]: wq/wk pair slice = [128, 1024]
    wq_sb = const.tile([128, NPAIR * CCH * 128], BF16, tag="wq")
    wk_sb = const.tile([128, NPAIR * CCH * 128], BF16, tag="wk")
    wv_sb = const.tile([128, CCH * DG], BF16, tag="wv")
    w_loaded = set()

    def load_wp(name, p):
        if (name, p) in w_loaded:
            return
        w_loaded.add((name, p))
        dst = {"q": wq_sb, "k": wk_sb}[name]
        src = {"q": wqP, "k": wkP}[name]
        nc.sync.dma_start(
            dst[:, p * CCH * 128 : (p + 1) * CCH * 128],
            src[128 * p : 128 * (p + 1), :],
        )

    def load_wv():
        if ("v", 0) in w_loaded:
            return
        w_loaded.add(("v", 0))
        nc.sync.dma_start(
            wv_sb[:].rearrange("p (c d) -> p c d", c=CCH),
            wvT[:].rearrange("(c p) d -> p c d", p=128),
        )

    def wqk(name, p, c):
        t = {"q": wq_sb, "k": wk_sb}[name]
        return t[:, p * CCH * 128 + c * 128 : p * CCH * 128 + (c + 1) * 128]

    # ---- persistent activations ----
    qt_sb = [
        persist.tile([128, S], F32R, tag=f"qt{p}", name=f"qt{p}")
        for p in range(NPAIR)
    ]
    kt_sb = [
        persist.tile([128, S], F32R, tag=f"kt{p}", name=f"kt{p}")
        for p in range(NPAIR)
    ]
    # V in [token, feature] layout with a ones column per (pair, chunk, head)
    v_sb = persist.tile([128, NPAIR * TCH * 2 * 65], BF16, tag="v")
    v_r = v_sb[:].rearrange("t (p c h d) -> t p c h d", p=NPAIR, c=TCH, h=2)
    nc.gpsimd.memset(v_r[:, :, :, :, 64:65], 1.0)

    # resident x_q^T [128, cch*S] bf16
    xq_res = persist.tile([128, CCH * S], BF16, tag="xq_res")
    xq_r = xq_res[:].rearrange("p (c t) -> p c t", c=CCH)

    # ---- projection units (generators; pumped between attention chunks) ----
    kx_tiles = {}
    vx_tiles = {}

    def x_load(which, tb):
        # one tile [128, (c t)] per token block, loaded in two half DMAs
        tiles, xT, tag = (
            (kx_tiles, xkT, "xk") if which == "k" else (vx_tiles, xvT, "xv")
        )
        xt = xpool.tile([128, CCH * 512], BF16, tag=tag, name=f"{tag}{tb}")
        xr = xt[:].rearrange("p (c t) -> p c t", c=CCH)
        tsl = slice(512 * tb, 512 * (tb + 1))
        for half in range(2):
            csl = slice(4 * half, 4 * half + 4)
            nc.sync.dma_start(
                xr[:, csl, :],
                xT[512 * half : 512 * (half + 1), tsl].rearrange(
                    "(c p) t -> p c t", p=128
                ),
            )
            yield
        tiles[tb] = xt

    def k_unit(p, tb):
        # yields only at points where no PSUM accumulation group is open:
        # a group whose matmuls get interleaved with foreign PE work
        # miscompiles on the real lowering path
        load_wp("k", p)
        yield
        if tb not in kx_tiles:
            yield from x_load("k", tb)
        xt = kx_tiles[tb][:].rearrange("p (c t) -> p c t", c=CCH)
        ps = psum.tile([128, 1024], F32, tag="sc", bufs=3, name="ps")
        for c in range(CCH):
            nc.tensor.matmul(
                ps[:, 0:512],
                wqk("k", p, c),
                xt[:, c, :],
                start=(c == 0),
                stop=(c == CCH - 1),
            )
        nc.vector.tensor_scalar_add(
            kt_sb[p][:, 512 * tb : 512 * (tb + 1)],
            ps[:, 0:512],
            bk_sb[:, p : p + 1],
        )

    def q_unit(p, tb):
        load_wp("q", p)
        yield
        ps = psum.tile([128, 1024], F32, tag="sc", bufs=3, name="ps")
        for c in range(CCH):
            nc.tensor.matmul(
                ps[:, 0:512],
                wqk("q", p, c),
                xq_r[:, c, 512 * tb : 512 * (tb + 1)],
                start=(c == 0),
                stop=(c == CCH - 1),
            )
        nc.vector.tensor_scalar_add(
            qt_sb[p][:, 512 * tb : 512 * (tb + 1)],
            ps[:, 0:512],
            bq_sb[:, p : p + 1],
        )

    def xq_load_unit(tb):
        tsl = slice(512 * tb, 512 * (tb + 1))
        for half in range(2):
            csl = slice(4 * half, 4 * half + 4)
            nc.sync.dma_start(
                xq_r[:, csl, tsl],
                xqT[512 * half : 512 * (half + 1), tsl].rearrange(
                    "(c p) t -> p c t", p=128
                ),
            )
            yield

    def v_unit(tb, j):
        load_wv()
        yield
        if tb not in vx_tiles:
            yield from x_load("v", tb)
        xt = vx_tiles[tb][:].rearrange("p (c t) -> p c t", c=CCH)
        c = 4 * tb + j
        t_sl = slice(128 * j, 128 * (j + 1))
        ps = psum.tile([128, 1024], F32, tag="sc", bufs=3, name="ps")
        for cc in range(CCH):
            nc.tensor.matmul(
                ps[:, 0:512],
                xt[:, cc, t_sl],
                wv_sb[:, cc * DG : (cc + 1) * DG],
                start=(cc == 0),
                stop=False,
            )
        nc.tensor.matmul(
            ps[:, 0:512],
            ones_row[:1, :],
            bv_sb[:1, :],
            start=False,
            stop=True,
        )
        nc.vector.tensor_copy(
            out=v_r[:, :, c, :, 0:64],
            in_=ps[:, 0:512].rearrange("t (p h d) -> t p h d", p=NPAIR, h=2),
        )

    def consts_unit():
        # issued on the ACT queue so these small transfers don't delay the
        # critical first weight/x loads on the sync queue
        nc.scalar.dma_start(cb_sb[:], cb[:])
        yield
        nc.scalar.dma_start(bv_sb[:], bv[:])
        yield

    units = []  # (g_due, seq, generator)
    seq_ctr = [0]

    def add_unit(g_due, gen):
        units.append((g_due, seq_ctr[0], gen))
        seq_ctr[0] += 1

    def build_units():
        add_unit(-1, consts_unit())
        for tb in range(TB):
            # K staggered tb-major so shared x tiles are consumed by
            # adjacent units (first-use order would deadlock xpool reuse
            # against the in-order DMA queue)
            for p in range(NPAIR):
                add_unit(4 * tb + p, k_unit(p, tb))
            if tb == 0:
                add_unit(0, xq_load_unit(tb))
            else:
                add_unit(16 * 4 * tb - 12, xq_load_unit(tb))
            for p in range(NPAIR):
                add_unit(16 * (4 * tb + p), q_unit(p, tb))
            for j in range(4):
                add_unit(4 * tb + j, v_unit(tb, j))
        units.sort(key=lambda t: (t[0], t[1]))

    pump_idx = [0]

    def pump(g, steps=None, margin=None):
        # HARD: fully emit every unit due <= g. Tile dependencies are
        # emission-ordered -- a consumer emitted before its producer reads
        # garbage -- so due units can never lag the attention stream.
        while pump_idx[0] < len(units) and units[pump_idx[0]][0] <= g:
            _, _, gen = units[pump_idx[0]]
            try:
                while True:
                    next(gen)
            except StopIteration:
                pump_idx[0] += 1
        # SOFT: advance up to `steps` sub-steps of soon-due units to smooth
        # PE filler work across the stream.
        n = 0
        cap = PUMP_STEPS if steps is None else steps
        mg = MARGIN if margin is None else margin
        while pump_idx[0] < len(units) and n < cap:
            due, _, gen = units[pump_idx[0]]
            if due > g + mg:
                break
            try:
                next(gen)
                n += 1
            except StopIteration:
                pump_idx[0] += 1

    def pump_all(g, margin):
        while pump_idx[0] < len(units) and units[pump_idx[0]][0] <= g + margin:
            due, _, gen = units[pump_idx[0]]
            try:
                while True:
                    next(gen)
            except StopIteration:
                pump_idx[0] += 1

    # ---- attention ----
    chunk_counter = [0]
    streams = [(h, j) for h in range(2) for j in range(4)]

    def emit_qk_exp(p, qb, c):
        q_sl = slice(512 * qb, 512 * (qb + 1))
        kt_sl = slice(128 * c, 128 * (c + 1))
        sc = psum.tile([128, 1024], F32, tag="sc", bufs=3)
        for h in (0, 1):
            hp_sl = slice(64 * h, 64 * (h + 1))
            nc.tensor.matmul(
                sc[:, 512 * h : 512 * (h + 1)],
                (kt_sb[p][hp_sl, kt_sl]),
                (qt_sb[p][hp_sl, q_sl]),
                start=True,
                stop=True,
            )
        pt = work.tile([128, 1024], BF16, tag="pt", bufs=6)
        gi = chunk_counter[0]
        chunk_counter[0] += 1
        if use_dve and (gi * dve_num) % dve_den < dve_num:
            y1 = y1pool.tile([128, 1024], F32, tag="y1", bufs=3)
            nc.vector._custom_dve(
                EXP_P1,
                out=y1[:],
                in0=sc[:],
                s0=_EXP_SCALE,
                s1=maskp_sb[:, c : c + 1],
                imm2=0.5,
            )
            nc.vector._custom_dve(EXP_P2, out=pt[:], in0=y1[:])
        else:
            nc.scalar.activation(
                pt[:], sc[:], Exp, bias=mask_sb[:, c : c + 1], scale=0.125
            )
        return pt

    def emit_pv(p, c, pt, ctxA, ctxB):
        for k, (h, j) in enumerate(streams):
            off = 512 * h + 128 * j
            if k < 7:
                dst = ctxA[:, 65 * k : 65 * k + 65]
                stop = c == TCH - 1 and k == 6
            else:
                dst = ctxB[:, 0:65]
                stop = c == TCH - 1
            nc.tensor.matmul(
                dst,
                (pt[:, off : off + 128]),
                (v_r[:, p, c, h, 0:65]),
                start=False,
                stop=stop,
                skip_group_check=True,
            )

    def emit_arms(ctxA, ctxB):
        # arm both banks: a zero outer-product covering every stream's bytes
        # (start=True) -- all stream matmuls then accumulate (start=False)
        # and stay order-independent even if the scheduler reorders them
        nc.tensor.matmul(
            ctxA[:, 0 : 7 * 65], zero_row[:1, 0:128], zero_row[:1, 0 : 7 * 65],
            start=True, stop=False, skip_group_check=True,
        )
        nc.tensor.matmul(
            ctxB[:, 0:65], zero_row[:1, 0:128], zero_row[:1, 0:65],
            start=True, stop=False, skip_group_check=True,
        )

    def emit_drains(p, qb, ctxA, ctxB):
        # drain ctx+den to DRAM via SBUF staging (ScalarE copies; DVE is
        # loaded with exp work); host normalizes
        st = stg.tile([128, NSTREAM * 65], F32, tag="st", name="st")
        nc.scalar.copy(st[:, 0 : 7 * 65], ctxA[:, 0 : 7 * 65])
        nc.scalar.copy(st[:, 7 * 65 : 8 * 65], ctxB[:, 0:65])
        row0 = (p * QB + qb) * 128
        nc.sync.dma_start(out[row0 : row0 + 128, :], st[:])

    # ---- phase order ----
    # PV lookahead carries ACROSS block boundaries: the previous block's last
    # PV chunks + its drains fill the PE while the next block's QK/exp ring
    # spins up; ctx banks are reused a full block later (no WAR stall).
    hp = os.environ.get("KHIPRI", "1") == "1"
    build_units()
    pump_all(-1, margin=1)  # prologue: consts, first K/Q/V units
    blocks = [(qb, p) for qb in range(QB) for p in range(NPAIR)]
    pend = []          # [(p, qb, c, pt, ctxA, ctxB)] cross-block PV queue
    armed = [None]     # ctx tiles of the block whose arms are pending

    def pop_pv():
        p0, qb0, c0, pt0, cA, cB = pend.pop(0)
        emit_pv(p0, c0, pt0, cA, cB)
        if c0 == TCH - 1:
            emit_drains(p0, qb0, cA, cB)
            if armed[0] is not None:
                emit_arms(*armed[0])
                armed[0] = None

    def run_block(i, p, qb):
        ctxA = psum.tile([128, 512], F32, tag="ctxA", name="ctxA")
        ctxB = psum.tile([128, 512], F32, tag="ctxB", name="ctxB")
        if i == 0:
            emit_arms(ctxA, ctxB)
        else:
            armed[0] = (ctxA, ctxB)  # armed after the previous block drains
        for c in range(TCH):
            pump(16 * i + c)
            pt = emit_qk_exp(p, qb, c)
            pend.append((p, qb, c, pt, ctxA, ctxB))
            while len(pend) > LOOK:
                pop_pv()

    for i, (qb, p) in enumerate(blocks):
        if hp:
            with tc.high_priority():
                run_block(i, p, qb)
        else:
            run_block(i, p, qb)
    while pend:
        pop_pv()
    pump_all(10 ** 9, margin=10 ** 9)  # flush any remaining units

    pools.close()


def make_in_maps(x_q, x_k, x_v, att_mask, W_q, b_q, W_k, b_k, W_v, b_v):
    import ml_dtypes

    f = np.float32
    bf = ml_dtypes.bfloat16
    x_q, x_k, x_v = (np.asarray(a, f) for a in (x_q, x_k, x_v))
    att_mask = np.asarray(att_mask, f)
    W_q, W_k, W_v = (np.asarray(a, f) for a in (W_q, W_k, W_v))
    b_q, b_k, b_v = (np.asarray(a, f) for a in (b_q, b_k, b_v))

    def pack_w(Wg):
        # Wg [DG out-features, H in] -> [NPAIR*128, CCH*128]:
        # row 128p+f (f = input-feature row within chunk), col c*128+d
        Wt = Wg.T.astype(bf)                       # [H in, DG out]
        r = Wt.reshape(CCH, 128, NPAIR, 128)       # [c, f, p, d]
        return np.ascontiguousarray(
            r.transpose(2, 1, 0, 3).reshape(NPAIR * 128, CCH * 128)
        )

    in_maps = []
    for core in range(NCORES):
        b, g = divmod(core, TP)
        fsl = slice(DG * g, DG * (g + 1))
        m = np.ascontiguousarray(att_mask[b, 0, 0].reshape(TCH, 128).T)
        cbv = np.concatenate(
            [
                b_q[fsl].reshape(NPAIR, 128).T,
                b_k[fsl].reshape(NPAIR, 128).T,
                m,
                (1.0 + m * np.float32(2.0 ** -11)).astype(f),
            ],
            axis=1,
        )
        in_maps.append(
            {
                "xqT": np.ascontiguousarray(x_q[b].T.astype(bf)),
                "xkT": np.ascontiguousarray(x_k[b].T.astype(bf)),
                "xvT": np.ascontiguousarray(x_v[b].T.astype(bf)),
                "wqP": pack_w(W_q[fsl, :]),
                "wkP": pack_w(W_k[fsl, :]),
                "wvT": np.ascontiguousarray(W_v[fsl, :].T.astype(bf)),
                "cb": np.ascontiguousarray(cbv.astype(f)),
                "bv": b_v[fsl].reshape(1, DG).astype(bf).copy(),
            }
        )
    return in_maps


def kernel(x_q, x_k, x_v, att_mask, W_q, b_q, W_k, b_k, W_v, b_v):
    global _CACHED
    if _CACHED is None:
        _CACHED = _build_core_program()
    nc = _CACHED

    in_maps = make_in_maps(
        x_q, x_k, x_v, att_mask, W_q, b_q, W_k, b_k, W_v, b_v
    )

    global LAST_RESULTS
    trace = TRACE or os.environ.get("BASS_KERNEL_TRACE", "") == "1"
    try:
        res = run_bass_kernel_spmd(nc, in_maps, list(range(NCORES)), trace=trace)
    except Exception:
        if not trace:
            raise
        res = run_bass_kernel_spmd(nc, in_maps, list(range(NCORES)))
    LAST_RESULTS = res

    # out rows are [(p, qb) blocks of 128 q] x [8 streams x 65]; stream
    # k=(h,j): global q = 512*qb + 128*j + r, feature = 128*p + 64*h + d.
    full = np.empty((B, S, H), np.float32)
    for core in range(NCORES):
        b, g = divmod(core, TP)
        r = res.results[core]["out"].reshape(NPAIR, QB, 128, 2, 4, 65)
        ctx = r[..., 0:64] / r[..., 64:65]        # [p, qb, r, h, j, d]
        # -> [qb, j, r, p, h, d] = [q, features-of-group]
        ctx = ctx.transpose(1, 4, 2, 0, 3, 5).reshape(S, DG)
        full[b, :, DG * g : DG * (g + 1)] = ctx
    return full


# revision 19
# speedup vs baseline: 1.2281x; 1.2281x over previous
"""Multi-head attention (B=4, S=2048, H=1024, NH=16) on 8 TRN2 NeuronCores.

Sharding: data-parallel over batch (4) x tensor-parallel over heads (2 groups
of 8 heads). Core c handles batch c//2, head-group c%2 (features 512*(c%2)..).
The host pre-transposes x to x^T [H, S] (bf16) and pre-packs W into the exact
SBUF layout [pair, 128, (chunk d)] so every weight load is one large
contiguous DMA; Q^T/K^T are kept in float32r (full PE rate, moving dim >=
256).

Per-core kernel:
  1. Projections run as fine-grained "units" (one PSUM accumulation group
     each) that are pumped matmul-by-matmul into the emission stream between
     attention chunks, so the in-order PE pipeline always has ready filler
     work while softmax-exp results are in flight.
  2. Attention per head-pair p (2 heads), 512-token q-block, 128-token
     kt-chunk:
       - two row-tiled QK^T matmuls produce S^T [128 kt, 512 q] per head,
       - exp(S^T/8 + mask) runs on EITHER ScalarE (activation) OR the DVE via
         a two-stage custom-DVE op (exp(t) = (((1+t*2^-11)^2+1)/2)^(2^11):
         base quadratic + 11 squarings split across two 8-stage uop passes),
         splitting the softmax-exp load across both engines,
       - PV matmuls are emitted in the [q, d] orientation: for each 128-token
         q-subtile j and head h, ctx[q, 0:64]+den[q] accumulates as
         pt[:, subtile]^T @ [V|ones] -- the 65-wide moving operand makes each
         accumulation step ~4x cheaper on the PE than the [d, q] orientation,
         and the softmax denominator still falls out of the ones column.
         PV emission trails QK by KLOOK chunks (software pipelining).
       - the 8 ctx streams of one (p, q-block) share two PSUM banks as one
         accumulation group each (PSUM pending-zero arms the whole 2KB region
         at the first start=True; each stream's first write then initializes
         its own bytes).
  3. ctx+den tiles drain via ScalarE copies to SBUF staging and DMA to DRAM;
     the host does the final normalization and layout (cheap numpy ops).
"""

import dataclasses
import os
from contextlib import ExitStack

import numpy as np

import concourse.mybir as mybir
import concourse.tile as tile
from concourse import bacc
from concourse.bass_utils import run_bass_kernel_spmd

B, S, H, NH, HD = 4, 2048, 1024, 16, 64
NCORES = 8
DP, TP = 4, 2            # batch-parallel x head-group-parallel
HG = NH // TP            # 8 heads per core
DG = HG * HD             # 512 features per core
NPAIR = HG // 2          # 4 head pairs (128 features each)
CCH = H // 128           # 8 contraction chunks for projections
TB = S // 512            # 4 token blocks of 512
TCH = S // 128           # 16 token chunks of 128
QB = S // 512            # 4 q-blocks of 512
NSTREAM = 8              # ctx accumulation streams per (pair, q-block)
F32 = mybir.dt.float32
F32R = mybir.dt.float32r
BF16 = mybir.dt.bfloat16

_CACHED = None
LAST_RESULTS = None   # BassKernelResults of the most recent run (for test.py)
TRACE = False         # set True (or BASS_KERNEL_TRACE=1) to profile the run

# ---------------------------------------------------------------------------
# Custom DVE exp: exp(t) ~= (0.5*(1 + t/2^11)^2 + 0.5)^(2^11), evaluated as
# pass1 = B(u)^(2^3) (base + 3 squarings, 8 ALU stages) and pass2 = x^(2^8)
# (8 squarings). Max rel err ~4e-4 in fp32 -- below bf16 output rounding.
# The mask enters as the per-partition scalar C1 = 1 + mask*2^-11.
# ---------------------------------------------------------------------------

_EXP_SCALE = float(0.125 * 2.0 ** -11)   # score scale 1/8 folded with 2^-11
_DVE_OPS = None


def _register_dve_exp():
    global _DVE_OPS
    if _DVE_OPS is not None:
        return _DVE_OPS
    import concourse.dve_ops as dve_ops
    from concourse.dve_ops import DveOp, _CUSTOM_DVE_ROW_BASE
    from concourse.dve_spec import Spec, Src0, C0, C1, C2, sq, lower
    from concourse.dve_spec import _has_src1 as has_src1
    from concourse.dve_uop import DveOpSpec

    def reg(name, spec):
        if name in dve_ops._SUB_OPCODE_FOR_NAME:
            return next(o for o in dve_ops.OPS if o.name == name)
        op = DveOp(name, spec, subdim=False, uops_sha={})
        dve_ops.OPS.append(op)
        dve_ops.CUSTOM_DVE_SPECS[name] = spec
        dve_ops._SUB_OPCODE_FOR_NAME[name] = _CUSTOM_DVE_ROW_BASE + len(dve_ops.OPS) - 1
        assert dve_ops._SUB_OPCODE_FOR_NAME[name] < 0x20, "opcode row overflow"
        shas = {}
        for ver in ("v3", "v4"):
            s = DveOpSpec(
                name=name,
                opcode=dve_ops._SUB_OPCODE_FOR_NAME[name],
                uops=lower(spec, ver=ver),
                rd1_en=has_src1(spec),
            )
            shas[ver] = s.sha(ver)
        op2 = dataclasses.replace(op, uops_sha=shas)
        dve_ops.OPS[-1] = op2
        return op2

    body1 = sq(Src0 * C0 + C1) * C2 + C2
    for _ in range(3):
        body1 = sq(body1)

    def ref1(in0, in1, s0, s1, imm2):
        x = in0.astype(np.float32)
        v = x * np.float32(s0) + np.float32(s1)
        y = ((v * v) * np.float32(imm2) + np.float32(imm2)).astype(np.float32)
        for _ in range(3):
            y = (y * y).astype(np.float32)
        return y

    body2 = Src0
    for _ in range(8):
        body2 = sq(body2)

    def ref2(in0, in1, s0, s1, imm2):
        y = in0.astype(np.float32)
        for _ in range(8):
            y = (y * y).astype(np.float32)
        return y

    p1 = reg("ANT_EXP_P1", Spec(body=body1, reference=ref1))
    p2 = reg("ANT_EXP_P2", Spec(body=body2, reference=ref2))
    _DVE_OPS = (p1, p2)
    return _DVE_OPS


def _build_core_program():
    nc = bacc.Bacc(
        "TRN2", target_bir_lowering=False, debug=False, enable_asserts=False
    )

    xqT = nc.declare_dram_parameter("xqT", [H, S], BF16, isOutput=False)
    xkT = nc.declare_dram_parameter("xkT", [H, S], BF16, isOutput=False)
    xvT = nc.declare_dram_parameter("xvT", [H, S], BF16, isOutput=False)
    # weights pre-packed per pair in SBUF layout: rows 128p+f, cols c*128+d
    wqP = nc.declare_dram_parameter("wqP", [NPAIR * 128, CCH * 128], BF16, isOutput=False)
    wkP = nc.declare_dram_parameter("wkP", [NPAIR * 128, CCH * 128], BF16, isOutput=False)
    # V weights stay [input-feature, output-feature] (moving operand is x)
    wvT = nc.declare_dram_parameter("wvT", [H, DG], BF16, isOutput=False)
    # combined per-partition constants: bq | bk | mask | maskp
    cb = nc.declare_dram_parameter("cb", [128, 2 * NPAIR + 2 * TCH], F32, isOutput=False)
    bv = nc.declare_dram_parameter("bv", [1, DG], BF16, isOutput=False)
    out = nc.declare_dram_parameter(
        "out", [NPAIR * QB * 128, NSTREAM * 65], F32, isOutput=True
    )

    with tile.TileContext(nc) as tc:
        _emit(tc, nc, xqT, xkT, xvT, wqP, wkP, wvT, cb, bv, out)

    nc.compile()
    return nc


def _emit(tc, nc, xqT, xkT, xvT, wqP, wkP, wvT, cb, bv, out):
    Exp = mybir.ActivationFunctionType.Exp

    use_dve = os.environ.get("KDVE", "1") == "1"
    dve_num = int(os.environ.get("KDVE_NUM", "4"))
    dve_den = int(os.environ.get("KDVE_DEN", "16"))
    LOOK = int(os.environ.get("KLOOK", "3"))
    MARGIN = int(os.environ.get("KMARGIN", "8"))
    PUMP_STEPS = int(os.environ.get("KPUMP", "3"))
    if use_dve:
        EXP_P1, EXP_P2 = _register_dve_exp()

    pools = ExitStack()
    const = pools.enter_context(tc.tile_pool(name="const", bufs=1))
    persist = pools.enter_context(tc.tile_pool(name="persist", bufs=1))
    xpool = pools.enter_context(tc.tile_pool(name="xpool", bufs=2))
    work = pools.enter_context(tc.tile_pool(name="work", bufs=6))
    y1pool = pools.enter_context(tc.tile_pool(name="y1pool", bufs=3))
    stg = pools.enter_context(tc.tile_pool(name="stg", bufs=2))
    psum = pools.enter_context(tc.tile_pool(name="psum", bufs=1, space="PSUM"))

    # ---- constants ----
    ones_row = const.tile([1, 128], BF16, tag="ones_row")
    nc.gpsimd.memset(ones_row[:], 1.0)
    zero_row = const.tile([1, 512], BF16, tag="zero_row")
    nc.gpsimd.memset(zero_row[:], 0.0)

    cb_sb = const.tile([128, 2 * NPAIR + 2 * TCH], F32, tag="cb")
    bq_sb = cb_sb[:, 0:NPAIR]
    bk_sb = cb_sb[:, NPAIR : 2 * NPAIR]
    mask_sb = cb_sb[:, 2 * NPAIR : 2 * NPAIR + TCH]
    maskp_sb = cb_sb[:, 2 * NPAIR + TCH : 2 * NPAIR + 2 * TCH]
    bv_sb = const.tile([1, DG], BF16, tag="bv")

    # ---- weights: per-pair lazy single-DMA loads ----
    # layout [128, (name-pair) * CCH*128]: wq/wk pair slice = [128, 1024]
    wq_sb = const.tile([128, NPAIR * CCH * 128], BF16, tag="wq")
    wk_sb = const.tile([128, NPAIR * CCH * 128], BF16, tag="wk")
    wv_sb = const.tile([128, CCH * DG], BF16, tag="wv")
    w_loaded = set()

    def load_wp(name, p):
        if (name, p) in w_loaded:
            return
        w_loaded.add((name, p))
        dst = {"q": wq_sb, "k": wk_sb}[name]
        src = {"q": wqP, "k": wkP}[name]
        nc.sync.dma_start(
            dst[:, p * CCH * 128 : (p + 1) * CCH * 128],
            src[128 * p : 128 * (p + 1), :],
        )

    def load_wv():
        if ("v", 0) in w_loaded:
            return
        w_loaded.add(("v", 0))
        nc.sync.dma_start(
            wv_sb[:].rearrange("p (c d) -> p c d", c=CCH),
            wvT[:].rearrange("(c p) d -> p c d", p=128),
        )

    def wqk(name, p, c):
        t = {"q": wq_sb, "k": wk_sb}[name]
        return t[:, p * CCH * 128 + c * 128 : p * CCH * 128 + (c + 1) * 128]

    # ---- persistent activations ----
    qt_sb = [
        persist.tile([128, S], F32R, tag=f"qt{p}", name=f"qt{p}")
        for p in range(NPAIR)
    ]
    kt_sb = [
        persist.tile([128, S], F32R, tag=f"kt{p}", name=f"kt{p}")
        for p in range(NPAIR)
    ]
    # V in [token, feature] layout with a ones column per (pair, chunk, head)
    v_sb = persist.tile([128, NPAIR * TCH * 2 * 65], BF16, tag="v")
    v_r = v_sb[:].rearrange("t (p c h d) -> t p c h d", p=NPAIR, c=TCH, h=2)
    nc.gpsimd.memset(v_r[:, :, :, :, 64:65], 1.0)

    # resident x_q^T [128, cch*S] bf16
    xq_res = persist.tile([128, CCH * S], BF16, tag="xq_res")
    xq_r = xq_res[:].rearrange("p (c t) -> p c t", c=CCH)

    # ---- projection units (generators; pumped between attention chunks) ----
    kx_tiles = {}
    vx_tiles = {}

    def x_load(which, tb):
        # one tile [128, (c t)] per token block, loaded in two half DMAs
        tiles, xT, tag = (
            (kx_tiles, xkT, "xk") if which == "k" else (vx_tiles, xvT, "xv")
        )
        xt = xpool.tile([128, CCH * 512], BF16, tag=tag, name=f"{tag}{tb}")
        xr = xt[:].rearrange("p (c t) -> p c t", c=CCH)
        tsl = slice(512 * tb, 512 * (tb + 1))
        for half in range(2):
            csl = slice(4 * half, 4 * half + 4)
            nc.sync.dma_start(
                xr[:, csl, :],
                xT[512 * half : 512 * (half + 1), tsl].rearrange(
                    "(c p) t -> p c t", p=128
                ),
            )
            yield
        tiles[tb] = xt

    def k_unit(p, tb):
        # yields only at points where no PSUM accumulation group is open:
        # a group whose matmuls get interleaved with foreign PE work
        # miscompiles on the real lowering path
        load_wp("k", p)
        yield
        if tb not in kx_tiles:
            yield from x_load("k", tb)
        xt = kx_tiles[tb][:].rearrange("p (c t) -> p c t", c=CCH)
        ps = psum.tile([128, 1024], F32, tag="sc", bufs=3, name="ps")
        for c in range(CCH):
            nc.tensor.matmul(
                ps[:, 0:512],
                wqk("k", p, c),
                xt[:, c, :],
                start=(c == 0),
                stop=(c == CCH - 1),
            )
        nc.vector.tensor_scalar_add(
            kt_sb[p][:, 512 * tb : 512 * (tb + 1)],
            ps[:, 0:512],
            bk_sb[:, p : p + 1],
        )

    def q_unit(p, tb):
        load_wp("q", p)
        yield
        ps = psum.tile([128, 1024], F32, tag="sc", bufs=3, name="ps")
        for c in range(CCH):
            nc.tensor.matmul(
                ps[:, 0:512],
                wqk("q", p, c),
                xq_r[:, c, 512 * tb : 512 * (tb + 1)],
                start=(c == 0),
                stop=(c == CCH - 1),
            )
        nc.vector.tensor_scalar_add(
            qt_sb[p][:, 512 * tb : 512 * (tb + 1)],
            ps[:, 0:512],
            bq_sb[:, p : p + 1],
        )

    def xq_load_unit(tb):
        tsl = slice(512 * tb, 512 * (tb + 1))
        for half in range(2):
            csl = slice(4 * half, 4 * half + 4)
            nc.sync.dma_start(
                xq_r[:, csl, tsl],
                xqT[512 * half : 512 * (half + 1), tsl].rearrange(
                    "(c p) t -> p c t", p=128
                ),
            )
            yield

    def v_unit(tb, j):
        load_wv()
        yield
        if tb not in vx_tiles:
            yield from x_load("v", tb)
        xt = vx_tiles[tb][:].rearrange("p (c t) -> p c t", c=CCH)
        c = 4 * tb + j
        t_sl = slice(128 * j, 128 * (j + 1))
        ps = psum.tile([128, 1024], F32, tag="sc", bufs=3, name="ps")
        for cc in range(CCH):
            nc.tensor.matmul(
                ps[:, 0:512],
                xt[:, cc, t_sl],
                wv_sb[:, cc * DG : (cc + 1) * DG],
                start=(cc == 0),
                stop=False,
            )
        nc.tensor.matmul(
            ps[:, 0:512],
            ones_row[:1, :],
            bv_sb[:1, :],
            start=False,
            stop=True,
        )
        nc.vector.tensor_copy(
            out=v_r[:, :, c, :, 0:64],
            in_=ps[:, 0:512].rearrange("t (p h d) -> t p h d", p=NPAIR, h=2),
        )

    def consts_unit():
        # issued on the ACT queue so these small transfers don't delay the
        # critical first weight/x loads on the sync queue
        nc.scalar.dma_start(cb_sb[:], cb[:])
        yield
        nc.scalar.dma_start(bv_sb[:], bv[:])
        yield

    units = []  # (g_due, seq, generator)
    seq_ctr = [0]

    def add_unit(g_due, gen):
        units.append((g_due, seq_ctr[0], gen))
        seq_ctr[0] += 1

    def build_units():
        add_unit(-1, consts_unit())
        for tb in range(TB):
            # K staggered tb-major so shared x tiles are consumed by
            # adjacent units (first-use order would deadlock xpool reuse
            # against the in-order DMA queue)
            for p in range(NPAIR):
                add_unit(4 * tb + p, k_unit(p, tb))
            if tb == 0:
                add_unit(0, xq_load_unit(tb))
            else:
                add_unit(16 * 4 * tb - 12, xq_load_unit(tb))
            for p in range(NPAIR):
                add_unit(16 * (4 * tb + p), q_unit(p, tb))
            for j in range(4):
                add_unit(4 * tb + j, v_unit(tb, j))
        units.sort(key=lambda t: (t[0], t[1]))

    pump_idx = [0]

    def pump(g, steps=None, margin=None):
        # HARD: fully emit every unit due <= g. Tile dependencies are
        # emission-ordered -- a consumer emitted before its producer reads
        # garbage -- so due units can never lag the attention stream.
        while pump_idx[0] < len(units) and units[pump_idx[0]][0] <= g:
            _, _, gen = units[pump_idx[0]]
            try:
                while True:
                    next(gen)
            except StopIteration:
                pump_idx[0] += 1
        # SOFT: advance up to `steps` sub-steps of soon-due units to smooth
        # PE filler work across the stream.
        n = 0
        cap = PUMP_STEPS if steps is None else steps
        mg = MARGIN if margin is None else margin
        while pump_idx[0] < len(units) and n < cap:
            due, _, gen = units[pump_idx[0]]
            if due > g + mg:
                break
            try:
                next(gen)
                n += 1
            except StopIteration:
                pump_idx[0] += 1

    def pump_all(g, margin):
        while pump_idx[0] < len(units) and units[pump_idx[0]][0] <= g + margin:
            due, _, gen = units[pump_idx[0]]
            try:
                while True:
                    next(gen)
            except StopIteration:
                pump_idx[0] += 1

    # ---- attention ----
    chunk_counter = [0]
    streams = [(h, j) for h in range(2) for j in range(4)]

    def emit_qk_exp(p, qb, c):
        q_sl = slice(512 * qb, 512 * (qb + 1))
        kt_sl = slice(128 * c, 128 * (c + 1))
        sc = psum.tile([128, 1024], F32, tag="sc", bufs=3)
        for h in (0, 1):
            hp_sl = slice(64 * h, 64 * (h + 1))
            nc.tensor.matmul(
                sc[:, 512 * h : 512 * (h + 1)],
                (kt_sb[p][hp_sl, kt_sl]),
                (qt_sb[p][hp_sl, q_sl]),
                start=True,
                stop=True,
            )
        pt = work.tile([128, 1024], BF16, tag="pt", bufs=6)
        gi = chunk_counter[0]
        chunk_counter[0] += 1
        if use_dve and (gi * dve_num) % dve_den < dve_num:
            y1 = y1pool.tile([128, 1024], F32, tag="y1", bufs=3)
            nc.vector._custom_dve(
                EXP_P1,
                out=y1[:],
                in0=sc[:],
                s0=_EXP_SCALE,
                s1=maskp_sb[:, c : c + 1],
                imm2=0.5,
            )
            nc.vector._custom_dve(EXP_P2, out=pt[:], in0=y1[:])
        else:
            nc.scalar.activation(
                pt[:], sc[:], Exp, bias=mask_sb[:, c : c + 1], scale=0.125
            )
        return pt

    def emit_pv(p, c, pt, ctxA, ctxB):
        for k, (h, j) in enumerate(streams):
            off = 512 * h + 128 * j
            if k < 7:
                dst = ctxA[:, 65 * k : 65 * k + 65]
                stop = c == TCH - 1 and k == 6
            else:
                dst = ctxB[:, 0:65]
                stop = c == TCH - 1
            nc.tensor.matmul(
                dst,
                (pt[:, off : off + 128]),
                (v_r[:, p, c, h, 0:65]),
                start=False,
                stop=stop,
                skip_group_check=True,
            )

    def emit_arms(ctxA, ctxB):
        # arm both banks: a zero outer-product covering every stream's bytes
        # (start=True) -- all stream matmuls then accumulate (start=False)
        # and stay order-independent even if the scheduler reorders them
        nc.tensor.matmul(
            ctxA[:, 0 : 7 * 65], zero_row[:1, 0:128], zero_row[:1, 0 : 7 * 65],
            start=True, stop=False, skip_group_check=True,
        )
        nc.tensor.matmul(
            ctxB[:, 0:65], zero_row[:1, 0:128], zero_row[:1, 0:65],
            start=True, stop=False, skip_group_check=True,
        )

    def emit_drains(p, qb, ctxA, ctxB):
        # drain ctx+den to SBUF staging split across ScalarE and DVE (in
        # parallel, ~0.5us each) and DMA to DRAM; host normalizes
        st = stg.tile([128, NSTREAM * 65], F32, tag="st", name="st")
        nc.scalar.copy(st[:, 0 : 4 * 65], ctxA[:, 0 : 4 * 65])
        nc.vector.tensor_copy(out=st[:, 4 * 65 : 7 * 65], in_=ctxA[:, 4 * 65 : 7 * 65])
        nc.vector.tensor_copy(out=st[:, 7 * 65 : 8 * 65], in_=ctxB[:, 0:65])
        row0 = (p * QB + qb) * 128
        nc.sync.dma_start(out[row0 : row0 + 128, :], st[:])

    # ---- phase order ----
    # PV lookahead carries ACROSS block boundaries: the previous block's last
    # PV chunks + its drains fill the PE while the next block's QK/exp ring
    # spins up; ctx banks are reused a full block later (no WAR stall).
    hp = os.environ.get("KHIPRI", "1") == "1"
    build_units()
    pump_all(-1, margin=1)  # prologue: consts, first K/Q/V units
    blocks = [(qb, p) for qb in range(QB) for p in range(NPAIR)]
    pend = []          # [(p, qb, c, pt, ctxA, ctxB)] cross-block PV queue
    armed = [None]     # ctx tiles of the block whose arms are pending

    def pop_pv():
        p0, qb0, c0, pt0, cA, cB = pend.pop(0)
        emit_pv(p0, c0, pt0, cA, cB)
        if c0 == TCH - 1:
            emit_drains(p0, qb0, cA, cB)
            if armed[0] is not None:
                emit_arms(*armed[0])
                armed[0] = None

    def run_block(i, p, qb):
        ctxA = psum.tile([128, 512], F32, tag="ctxA", name="ctxA")
        ctxB = psum.tile([128, 512], F32, tag="ctxB", name="ctxB")
        if i == 0:
            emit_arms(ctxA, ctxB)
        else:
            armed[0] = (ctxA, ctxB)  # armed after the previous block drains
        for c in range(TCH):
            pump(16 * i + c)
            pt = emit_qk_exp(p, qb, c)
            pend.append((p, qb, c, pt, ctxA, ctxB))
            while len(pend) > LOOK:
                pop_pv()

    for i, (qb, p) in enumerate(blocks):
        if hp:
            with tc.high_priority():
                run_block(i, p, qb)
        else:
            run_block(i, p, qb)
    while pend:
        pop_pv()
    pump_all(10 ** 9, margin=10 ** 9)  # flush any remaining units

    pools.close()


def make_in_maps(x_q, x_k, x_v, att_mask, W_q, b_q, W_k, b_k, W_v, b_v):
    import ml_dtypes

    f = np.float32
    bf = ml_dtypes.bfloat16
    x_q, x_k, x_v = (np.asarray(a, f) for a in (x_q, x_k, x_v))
    att_mask = np.asarray(att_mask, f)
    W_q, W_k, W_v = (np.asarray(a, f) for a in (W_q, W_k, W_v))
    b_q, b_k, b_v = (np.asarray(a, f) for a in (b_q, b_k, b_v))

    def pack_w(Wg):
        # Wg [DG out-features, H in] -> [NPAIR*128, CCH*128]:
        # row 128p+f (f = input-feature row within chunk), col c*128+d
        Wt = Wg.T.astype(bf)                       # [H in, DG out]
        r = Wt.reshape(CCH, 128, NPAIR, 128)       # [c, f, p, d]
        return np.ascontiguousarray(
            r.transpose(2, 1, 0, 3).reshape(NPAIR * 128, CCH * 128)
        )

    in_maps = []
    for core in range(NCORES):
        b, g = divmod(core, TP)
        fsl = slice(DG * g, DG * (g + 1))
        m = np.ascontiguousarray(att_mask[b, 0, 0].reshape(TCH, 128).T)
        cbv = np.concatenate(
            [
                b_q[fsl].reshape(NPAIR, 128).T,
                b_k[fsl].reshape(NPAIR, 128).T,
                m,
                (1.0 + m * np.float32(2.0 ** -11)).astype(f),
            ],
            axis=1,
        )
        in_maps.append(
            {
                "xqT": np.ascontiguousarray(x_q[b].T.astype(bf)),
                "xkT": np.ascontiguousarray(x_k[b].T.astype(bf)),
                "xvT": np.ascontiguousarray(x_v[b].T.astype(bf)),
                "wqP": pack_w(W_q[fsl, :]),
                "wkP": pack_w(W_k[fsl, :]),
                "wvT": np.ascontiguousarray(W_v[fsl, :].T.astype(bf)),
                "cb": np.ascontiguousarray(cbv.astype(f)),
                "bv": b_v[fsl].reshape(1, DG).astype(bf).copy(),
            }
        )
    return in_maps


def kernel(x_q, x_k, x_v, att_mask, W_q, b_q, W_k, b_k, W_v, b_v):
    global _CACHED
    if _CACHED is None:
        _CACHED = _build_core_program()
    nc = _CACHED

    in_maps = make_in_maps(
        x_q, x_k, x_v, att_mask, W_q, b_q, W_k, b_k, W_v, b_v
    )

    global LAST_RESULTS
    trace = TRACE or os.environ.get("BASS_KERNEL_TRACE", "") == "1"
    try:
        res = run_bass_kernel_spmd(nc, in_maps, list(range(NCORES)), trace=trace)
    except Exception:
        if not trace:
            raise
        res = run_bass_kernel_spmd(nc, in_maps, list(range(NCORES)))
    LAST_RESULTS = res

    # out rows are [(p, qb) blocks of 128 q] x [8 streams x 65]; stream
    # k=(h,j): global q = 512*qb + 128*j + r, feature = 128*p + 64*h + d.
    full = np.empty((B, S, H), np.float32)
    for core in range(NCORES):
        b, g = divmod(core, TP)
        r = res.results[core]["out"].reshape(NPAIR, QB, 128, 2, 4, 65)
        ctx = r[..., 0:64] / r[..., 64:65]        # [p, qb, r, h, j, d]
        # -> [qb, j, r, p, h, d] = [q, features-of-group]
        ctx = ctx.transpose(1, 4, 2, 0, 3, 5).reshape(S, DG)
        full[b, :, DG * g : DG * (g + 1)] = ctx
    return full
